# revision 64
# baseline (speedup 1.0000x reference)
"""Trainium2 Bass kernel for nn_Deformation (5-head dense MLP, N=200k points).

Strategy (pure data parallel, sharding N across 8 cores; ~402us HW exec):
  - Host precomputes the concatenated "hidden" vector per point, applies
    the leading ReLU, and lays it out feature-major ([feature, point]) so
    the on-chip MLP never transposes. The 64 time-positional-encoding rows
    are constant across points (they depend only on time_emb[0,0]) and get
    folded into the layer-1 bias in float64, shrinking K to 192.
  - On chip, per 512-point tile: 5 head layer-1 matmuls (K=192 as 128+64
    chunks, M=256 as 2x128, fp32r at full PE rate), bias+ReLU drains split
    across ScalarE/VectorE, a single block-diagonal layer-2 matmul
    (K=5*256, M=59 = all five head outputs at once), sigmoid mask, mask
    broadcast across partitions via a K=1 matmul against a 0/1 row vector,
    and the masked residual adds on VectorE.
  - The epilogue (bcast matmul + adds + output DMAs) is emitted one tile
    behind the matmuls so the PE never waits on the ACT sigmoid, and
    output DMAs ride gpsimd/SWDGE so they never block input prefetch on
    the in-order sync queue. PE stays gap-free and HAM-warm throughout.
  - Outputs are written feature-major; the host transposes/splits into
    the 7 reference outputs. opacity is a passthrough of the input.
"""

import numpy as np
from contextlib import ExitStack

import concourse.bass as bass
import concourse.mybir as mybir
import concourse.tile as tile
from concourse import bacc, bass_utils

F32 = mybir.dt.float32
F32R = mybir.dt.float32r
AF = mybir.ActivationFunctionType
ALU = mybir.AluOpType

N_TOTAL = 200000
N_CORES = 8
N_PER_CORE = N_TOTAL // N_CORES  # 25000
W = 255
WP = 256                      # padded width
TILE_N = 512
TILES_PER_CORE = 49           # 49*512 = 25088 >= 25000
NPAD = TILE_N * TILES_PER_CORE
M2 = 59                       # mask(1) + pos(3) + scl(3) + rot(4) + shs(48)
NHEADS = 5
K2 = 2 * NHEADS               # layer-2 K chunks (5 heads x 2 chunks of 128)

# rows of the hidden vector (concat order from the reference). The 64 tpe
# rows (191:255) are constant across points and get folded into the layer-1
# bias on the host, so the on-chip contraction is K=191 padded to 192.
HEAD_NAMES = ["mask", "pos", "scl", "rot", "shs"]
HEAD_OUT = [1, 3, 3, 4, 48]
# layer-2 output rows: mask 0, pos 1:4, scl 4:7, rot 7:11, shs 11:59
KIN = 191          # non-constant hidden features
K1P = 192          # padded on-chip contraction for layer 1 (3 x K=64 chunks)

# packed constants, split hot (layer-1 weights/biases) / cold (the rest)
# into two SBUF tiles so layer 1 only depends on the first DMA.
# hot region (one SBUF tile):
CP_W1A = 0                                  # 10 x [128,128]: (h,m) K-rows 0:128
CP_W1BF = CP_W1A + 2 * NHEADS * 128         # 10 x [128,128]: (h,m) K-rows 128:192 at
                                            #   partitions 0:64, zeros at 64:128
CP_B1 = CP_W1BF + 2 * NHEADS * 128          # [128, 10] f32 layer-1 biases (tpe folded)
HOT_COLS = CP_B1 + 16
# cold region (second SBUF tile; offsets within it):
CC_W2 = 0                                   # [128, 10*59] f32r block-diag layer2
CC_B2 = CC_W2 + K2 * M2                     # rows 0:59 of one col: f32 layer-2 bias
CC_ONES = CC_B2 + 8                         # row 0: [0, 1 x 58] bcast vector
CC_W1B = CC_ONES + 64                       # 5 x [128,128] row-tiled tails (unused
                                            #   unless ROW_TILED)
COLD_COLS = CC_W1B + NHEADS * 128
CPACK_COLS = HOT_COLS + COLD_COLS


# Concurrent 64-row PE tiling (tile_position T0/T8 pairs) wedges the device on
# this stack (NRT_EXEC_UNIT_UNRECOVERABLE), so layer 1 uses full-array matmuls
# with a zero-padded K=64 tail chunk instead. Code path kept for reference.
ROW_TILED = False
# Broadcast the sigmoid mask across partitions on the (otherwise idle) GpSimd
# engine instead of a K=1 PE matmul — saves one PE matmul per tile.
GP_BCAST = True


def build_bass(npad=NPAD, tile_n=TILE_N, mm_dtype=F32R, row_tiled=ROW_TILED):
    """Build the per-core Bass module (same NEFF on all cores)."""
    ntiles = npad // tile_n
    nc = bacc.Bacc("TRN2", target_bir_lowering=False, debug=False)

    hr = nc.dram_tensor("hr", [128, 2 * npad], mm_dtype, kind="ExternalInput").ap()
    raw = nc.dram_tensor("raw", [M2, npad], F32, kind="ExternalInput").ap()
    cpack = nc.dram_tensor("cpack", [128, CPACK_COLS], mm_dtype,
                           kind="ExternalInput").ap()
    outA = nc.dram_tensor("outA", [M2 - 1, npad], F32, kind="ExternalOutput").ap()
    outB = nc.dram_tensor("outB", [M2 - 1, npad], F32, kind="ExternalOutput").ap()

    with tile.TileContext(nc) as tc, ExitStack() as ctx:
        const = ctx.enter_context(tc.tile_pool(name="const", bufs=1))
        hrp = ctx.enter_context(tc.tile_pool(name="hrp", bufs=4))
        h1p = ctx.enter_context(tc.tile_pool(name="h1p", bufs=2))
        epi = ctx.enter_context(tc.tile_pool(name="epi", bufs=3))
        ps1p = ctx.enter_context(tc.tile_pool(name="ps1p", bufs=6, space="PSUM"))
        ps2p = ctx.enter_context(tc.tile_pool(name="ps2p", bufs=2, space="PSUM"))
        bcp = ctx.enter_context(tc.tile_pool(name="bcp", bufs=1, space="PSUM"))

        cph = const.tile([128, HOT_COLS], mm_dtype, tag="cph")
        nc.sync.dma_start(cph[:], cpack[:, 0:HOT_COLS])
        cpc = const.tile([128, COLD_COLS], mm_dtype, tag="cpc")
        # (cold-region DMA is emitted after tile 0's input DMAs below, so the
        # first tile's prefetch isn't queued behind it on the in-order SP)

        def w1a_ap(h, m, slot):
            c = CP_W1A + (h * 2 + m) * 128
            return cph[slot * 64:(slot + 1) * 64, c:c + 128]

        def w1b_ap(h, slot):
            c = CC_W1B + h * 128
            return cpc[slot * 64:(slot + 1) * 64, c:c + 128]

        def w1bf_ap(h, m):
            c = CP_W1BF + (h * 2 + m) * 128
            return cph[:, c:c + 128]

        def w2_ap(kc):
            c = CC_W2 + kc * M2
            return cpc[:, c:c + M2]

        def b1_ap(j):
            return cph[:, CP_B1 + j:CP_B1 + j + 1].bitcast(F32)

        b2_all = cpc[0:M2, CC_B2:CC_B2 + 1].bitcast(F32)
        b2_sig = cpc[0:1, CC_B2:CC_B2 + 1].bitcast(F32)
        ones_ap = cpc[0:1, CC_ONES:CC_ONES + M2]  # mm_dtype, for the bcast matmul

        def epilogue(pend):
            """Masked adds + output DMAs for a finished tile. Emitted one tile
            late so the mask broadcast never waits on the sigmoid, and the
            output DMAs (on gpsimd/SWDGE) never block input prefetch."""
            mask_sb, delt, raw_sb, c0, c1 = pend
            if GP_BCAST:
                bc = epi.tile([M2, tile_n], F32, tag="bc")
                nc.gpsimd.partition_broadcast(bc[:], mask_sb[:], channels=M2)
            else:
                bc = bcp.tile([M2, tile_n], F32, tag="bc")
                nc.tensor.matmul(bc[:], ones_ap, mask_sb[:], start=True, stop=True)
            msk = epi.tile([M2, tile_n], F32, tag="msk")
            nc.vector.tensor_mul(msk[:], delt[:], bc[:])
            outs = epi.tile([M2, tile_n], F32, tag="outs")
            nc.vector.tensor_add(outs[:], msk[:], raw_sb[:])
            nc.gpsimd.dma_start(outA[:, c0:c1], outs[1:M2, :])
            nc.gpsimd.dma_start(outB[:, c0:c1], delt[1:M2, :])

        pend = None
        hr3 = hr.rearrange("p (kc n) -> p kc n", kc=2)
        for t in range(ntiles):
            c0, c1 = t * tile_n, (t + 1) * tile_n

            hrt = hrp.tile([128, 2, tile_n], mm_dtype, tag="hrt")
            nc.sync.dma_start(hrt[:], hr3[:, :, c0:c1])

            raw_sb = epi.tile([M2, tile_n], F32, tag="raw")
            nc.sync.dma_start(raw_sb[:], raw[:, c0:c1])
            if t == 0:
                nc.sync.dma_start(cpc[:], cpack[:, HOT_COLS:CPACK_COLS])

            h1 = h1p.tile([128, K2 * tile_n], mm_dtype, tag="h1")
            ps2 = ps2p.tile([M2, tile_n], F32, tag="ps2")

            def l2_mm(kc):
                nc.tensor.matmul(
                    ps2[:], w2_ap(kc),
                    h1[:, kc * tile_n:(kc + 1) * tile_n],
                    start=(kc == 0), stop=(kc == K2 - 1))

            for h in range(NHEADS):
                ps_m0 = ps1p.tile([128, tile_n], F32, tag="ps1")
                ps_m1 = ps1p.tile([128, tile_n], F32, tag="ps1")
                ps = [ps_m0, ps_m1]
                if row_tiled:
                    # 6 K=64 matmuls per head, strictly alternating T0/T8:
                    # m0-c0@T0, m0-c1@T8, m1-c0@T0, m1-c1@T8, m0-c2@T0, m1-c2@T8
                    for m in range(2):
                        nc.tensor.matmul(ps[m][:], w1a_ap(h, m, 0),
                                         hrt[0:64, 0, :], start=True, stop=False)
                        nc.tensor.matmul(ps[m][:], w1a_ap(h, m, 1),
                                         hrt[64:128, 0, :], start=False, stop=False)
                    for m in range(2):
                        nc.tensor.matmul(ps[m][:], w1b_ap(h, m),
                                         hrt[m * 64:(m + 1) * 64, 1, :],
                                         start=False, stop=True)
                else:
                    # full-array: K=128 chunk + zero-padded K=64 tail chunk
                    for m in range(2):
                        c = CP_W1A + (h * 2 + m) * 128
                        nc.tensor.matmul(ps[m][:], cph[:, c:c + 128],
                                         hrt[:, 0, :], start=True, stop=False)
                        nc.tensor.matmul(ps[m][:], w1bf_ap(h, m), hrt[:, 1, :],
                                         start=False, stop=True)
                for m in range(2):
                    j = 2 * h + m
                    h1s = h1[:, j * tile_n:(j + 1) * tile_n]
                    if m == 0:
                        nc.vector.tensor_scalar(
                            h1s, ps[m][:], b1_ap(j), 0.0, ALU.add, ALU.max)
                    else:
                        nc.scalar.activation(h1s, ps[m][:], AF.Relu,
                                             bias=b1_ap(j))
                # interleave layer-2 matmuls two heads behind their drains so
                # the PE never reaches an h1 block before ACT/DVE finished it
                if h >= 2:
                    l2_mm(2 * (h - 2))
                    l2_mm(2 * (h - 2) + 1)
            for kc in range(2 * (NHEADS - 2), K2):
                l2_mm(kc)

            mask_sb = epi.tile([1, tile_n], F32 if GP_BCAST else mm_dtype,
                               tag="mask")
            nc.scalar.activation(mask_sb[:], ps2[0:1, :], AF.Sigmoid, bias=b2_sig)
            # full-range drain (row 0 = mask logit + bias, unused downstream)
            delt = epi.tile([M2, tile_n], F32, tag="delt")
            nc.scalar.activation(delt[:], ps2[:], AF.Identity, bias=b2_all)

            if pend is not None:
                epilogue(pend)
            pend = (mask_sb, delt, raw_sb, c0, c1)
        epilogue(pend)

    nc.compile()
    return nc


def _time_pos_enc(t):
    i = np.arange(0, 64, 2, dtype=np.float32)
    freq = np.power(np.float32(10000.0), i / np.float32(32.0)).astype(np.float32)
    t = np.float32(t)
    return np.stack([np.sin(t / freq), np.cos(t / freq)], axis=1).reshape(64)


def prep_weights(inputs):
    """Pack weights/biases into the kernel's SBUF-ready layouts.

    The 64 constant tpe rows of the hidden vector are folded into the
    layer-1 bias here (in float64), shrinking the on-chip K to 192."""
    tpe = _time_pos_enc(np.asarray(inputs["time_emb"], np.float32)[0, 0])
    tpe_relu = np.maximum(tpe, 0).astype(np.float64)

    b1k = np.zeros((NHEADS, 2, 128), dtype=np.float32)
    w2stack = np.zeros((K2 * 128, M2), dtype=np.float32)
    b2k = np.zeros((M2, 1), dtype=np.float32)
    cpk = np.zeros((128, CPACK_COLS), dtype=np.float32)
    col = 0
    for hi, hn in enumerate(HEAD_NAMES):
        w1_h = np.asarray(inputs[f"{hn}_w1"], np.float32)
        b1_h = np.asarray(inputs[f"{hn}_b1"], np.float32)
        w2_h = np.asarray(inputs[f"{hn}_w2"], np.float32)
        b2_h = np.asarray(inputs[f"{hn}_b2"], np.float32)
        oh = HEAD_OUT[hi]
        # fold tpe rows (191:255) into the bias, pad K 191->192 and M 255->256
        b1_eff = (b1_h.astype(np.float64)
                  + tpe_relu @ w1_h[KIN:W, :].astype(np.float64))
        w1p = np.zeros((K1P, WP), dtype=np.float32)
        w1p[:KIN, :W] = w1_h[:KIN, :]
        for m in range(2):
            mc = slice(m * 128, (m + 1) * 128)
            cpk[:, CP_W1A + (hi * 2 + m) * 128:
                CP_W1A + (hi * 2 + m + 1) * 128] = w1p[0:128, mc]
            c = HOT_COLS + CC_W1B + hi * 128
            cpk[m * 64:(m + 1) * 64, c:c + 128] = w1p[128:K1P, mc]
            cf = CP_W1BF + (hi * 2 + m) * 128
            cpk[0:64, cf:cf + 128] = w1p[128:K1P, mc]
        b1k[hi] = np.concatenate(
            [b1_eff, [0.0]]).astype(np.float32).reshape(2, 128)
        w2stack[hi * WP:hi * WP + W, col:col + oh] = w2_h
        b2k[col:col + oh, 0] = b2_h
        col += oh
    assert col == M2
    w2_out = np.ascontiguousarray(
        w2stack.reshape(K2, 128, M2).transpose(1, 0, 2).reshape(128, K2 * M2))
    b1_out = np.ascontiguousarray(
        b1k.transpose(2, 0, 1).reshape(128, NHEADS * 2))

    cpk[:, HOT_COLS + CC_W2:HOT_COLS + CC_W2 + K2 * M2] = w2_out
    cpk[:, CP_B1:CP_B1 + 2 * NHEADS] = b1_out
    cpk[0:M2, HOT_COLS + CC_B2] = b2k[:, 0]
    cpk[0, HOT_COLS + CC_ONES + 1:HOT_COLS + CC_ONES + M2] = 1.0
    return cpk


def prep_points(inputs, npad_total):
    """Build relu(hidden) feature-major [256, npad_total] and raw [58, npad_total]."""
    point = np.asarray(inputs["point"], np.float32)
    n = point.shape[0]
    if npad_total is None:
        npad_total = n
    rot = np.asarray(inputs["rotations_input"], np.float32)
    scl = np.asarray(inputs["scales_input"], np.float32)
    opac = np.asarray(inputs["opacity_emb"], np.float32)
    shs = np.asarray(inputs["shs_emb"], np.float32).reshape(n, 48)
    sem = np.asarray(inputs["semantic_feature"], np.float32)
    dx = np.asarray(inputs["dx"], np.float32)
    temb = np.asarray(inputs["time_emb"], np.float32)

    # tpe rows are folded into the layer-1 bias host-side; only the 191
    # per-point features ship to the device (padded to 192)
    hr = np.zeros((K1P, npad_total), dtype=np.float32)
    hr[0:3, :n] = np.maximum(point, 0).T
    hr[3:7, :n] = np.maximum(rot, 0).T
    hr[7:10, :n] = np.maximum(scl, 0).T
    hr[10:11, :n] = np.maximum(opac, 0).T
    hr[11:59, :n] = np.maximum(shs, 0).T
    hr[59:187, :n] = np.maximum(sem, 0).T
    hr[187:190, :n] = np.maximum(dx, 0).T
    hr[190:191, :n] = np.maximum(temb, 0).T

    raw = np.zeros((M2, npad_total), dtype=np.float32)
    raw[1:4, :n] = point.T
    raw[4:7, :n] = scl.T
    raw[7:11, :n] = rot.T
    raw[11:59, :n] = shs.T
    return hr, raw


_CACHED = {}


def _get_nc():
    if "nc" not in _CACHED:
        _CACHED["nc"] = build_bass()
    return _CACHED["nc"]


def make_in_maps(inputs, npad=NPAD, n_cores=N_CORES):
    """Shard + pack full inputs into per-core kernel input dicts."""
    cpk = prep_weights(inputs)
    hr_full, raw_full = prep_points(inputs, None)
    n = hr_full.shape[1]
    n_per_core = n // n_cores
    assert n_per_core * n_cores == n and n_per_core <= npad

    in_maps = []
    for c in range(n_cores):
        s = c * n_per_core
        hrc = np.zeros((K1P, npad), dtype=np.float32)
        hrc[:, :n_per_core] = hr_full[:, s:s + n_per_core]
        rawc = np.zeros((M2, npad), dtype=np.float32)
        rawc[:, :n_per_core] = raw_full[:, s:s + n_per_core]
        # kc=0: hidden rows 0:128; kc=1: rows 128:192 duplicated across both
        # 64-partition halves (feeds the row-tiled K=64 tail matmuls)
        bot = np.concatenate([hrc[128:K1P], hrc[128:K1P]], axis=0)
        hrk = np.ascontiguousarray(
            np.stack([hrc[0:128], bot], axis=1).reshape(128, 2 * npad))
        in_maps.append({"hr": hrk, "raw": rawc, "cpack": cpk})
    return in_maps


def assemble_outputs(inputs, outA, outB, n):
    """outA/outB: [58, n] feature-major device outputs -> reference 7-tuple."""
    pts = np.ascontiguousarray(outA[0:3].T)
    scales = np.ascontiguousarray(outA[3:6].T)
    rotations = np.ascontiguousarray(outA[6:10].T)
    shs_out = np.ascontiguousarray(outA[10:58].T).reshape(n, 16, 3)
    dx_out = np.ascontiguousarray(outB[0:3].T)
    dshs = np.ascontiguousarray(outB[10:58].T).reshape(n, 16, 3)
    opacity = np.asarray(inputs["opacity_emb"], np.float32)[:, :1].copy()
    return (pts, scales, rotations, opacity, shs_out, dx_out, dshs)


def kernel(**inputs):
    import os
    nc = _get_nc()
    in_maps = make_in_maps(inputs)
    trace = bool(int(os.environ.get("KERNEL_TRACE", "0")))

    res = bass_utils.run_bass_kernel_spmd(
        nc, in_maps, core_ids=list(range(N_CORES)), trace=trace)
    _CACHED["last_results"] = res

    outA = np.concatenate(
        [res.results[c]["outA"][:, :N_PER_CORE] for c in range(N_CORES)], axis=1)
    outB = np.concatenate(
        [res.results[c]["outB"][:, :N_PER_CORE] for c in range(N_CORES)], axis=1)
    return assemble_outputs(inputs, outA, outB, N_TOTAL)


# revision 67
# speedup vs baseline: 1.0362x; 1.0362x over previous
"""Trainium2 Bass kernel for nn_Deformation (5-head dense MLP, N=200k points).

Strategy (pure data parallel, sharding N across 8 cores; ~402us HW exec):
  - Host precomputes the concatenated "hidden" vector per point, applies
    the leading ReLU, and lays it out feature-major ([feature, point]) so
    the on-chip MLP never transposes. The 64 time-positional-encoding rows
    are constant across points (they depend only on time_emb[0,0]) and get
    folded into the layer-1 bias in float64, shrinking K to 192.
  - On chip, per 512-point tile: 5 head layer-1 matmuls (K=192 as 128+64
    chunks, M=256 as 2x128, fp32r at full PE rate), bias+ReLU drains split
    across ScalarE/VectorE, a single block-diagonal layer-2 matmul
    (K=5*256, M=59 = all five head outputs at once), sigmoid mask, mask
    broadcast across partitions via a K=1 matmul against a 0/1 row vector,
    and the masked residual adds on VectorE.
  - The epilogue (bcast matmul + adds + output DMAs) is emitted one tile
    behind the matmuls so the PE never waits on the ACT sigmoid, and
    output DMAs ride gpsimd/SWDGE so they never block input prefetch on
    the in-order sync queue. PE stays gap-free and HAM-warm throughout.
  - Outputs are written feature-major; the host transposes/splits into
    the 7 reference outputs. opacity is a passthrough of the input.
"""

import numpy as np
from contextlib import ExitStack

import concourse.bass as bass
import concourse.mybir as mybir
import concourse.tile as tile
from concourse import bacc, bass_utils

F32 = mybir.dt.float32
F32R = mybir.dt.float32r
AF = mybir.ActivationFunctionType
ALU = mybir.AluOpType

N_TOTAL = 200000
N_CORES = 8
N_PER_CORE = N_TOTAL // N_CORES  # 25000
W = 255
WP = 256                      # padded width
TILE_N = 512
TILES_PER_CORE = 49           # 49*512 = 25088 >= 25000
NPAD = TILE_N * TILES_PER_CORE
M2 = 59                       # mask(1) + pos(3) + scl(3) + rot(4) + shs(48)
NHEADS = 5
K2 = 2 * NHEADS               # layer-2 K chunks (5 heads x 2 chunks of 128)

# rows of the hidden vector (concat order from the reference). The 64 tpe
# rows (191:255) are constant across points and get folded into the layer-1
# bias on the host, so the on-chip contraction is K=191 padded to 192.
HEAD_NAMES = ["mask", "pos", "scl", "rot", "shs"]
HEAD_OUT = [1, 3, 3, 4, 48]
# layer-2 output rows: mask 0, pos 1:4, scl 4:7, rot 7:11, shs 11:59
KIN = 191          # non-constant hidden features
K1P = 192          # padded on-chip contraction for layer 1 (3 x K=64 chunks)

# packed constants, split hot (layer-1 weights/biases) / cold (the rest)
# into two SBUF tiles so layer 1 only depends on the first DMA.
# hot region (one SBUF tile):
CP_W1A = 0                                  # 10 x [128,128]: (h,m) K-rows 0:128
CP_W1BF = CP_W1A + 2 * NHEADS * 128         # 10 x [128,128]: (h,m) K-rows 128:192 at
                                            #   partitions 0:64, zeros at 64:128
CP_B1 = CP_W1BF + 2 * NHEADS * 128          # [128, 10] f32 layer-1 biases (tpe folded)
HOT_COLS = CP_B1 + 16
# cold region (second SBUF tile; offsets within it):
CC_W2 = 0                                   # [128, 10*59] f32r block-diag layer2
CC_B2 = CC_W2 + K2 * M2                     # rows 0:59 of one col: f32 layer-2 bias
CC_ONES = CC_B2 + 8                         # row 0: [0, 1 x 58] bcast vector
CC_W1B = CC_ONES + 64                       # 5 x [128,128] row-tiled tails (unused
                                            #   unless ROW_TILED)
COLD_COLS = CC_W1B + NHEADS * 128
CPACK_COLS = HOT_COLS + COLD_COLS


# Concurrent 64-row PE tiling (tile_position T0/T8 pairs) wedges the device on
# this stack (NRT_EXEC_UNIT_UNRECOVERABLE), so layer 1 uses full-array matmuls
# with a zero-padded K=64 tail chunk instead. Code path kept for reference.
ROW_TILED = False
# Broadcast the sigmoid mask across partitions on the (otherwise idle) GpSimd
# engine instead of a K=1 PE matmul — saves one PE matmul per tile.
GP_BCAST = True


def build_bass(npad=NPAD, tile_n=TILE_N, mm_dtype=F32R, row_tiled=ROW_TILED):
    """Build the per-core Bass module (same NEFF on all cores)."""
    ntiles = npad // tile_n
    nc = bacc.Bacc("TRN2", target_bir_lowering=False, debug=False)

    hr = nc.dram_tensor("hr", [128, 2 * npad], mm_dtype, kind="ExternalInput").ap()
    raw = nc.dram_tensor("raw", [M2, npad], F32, kind="ExternalInput").ap()
    cpack = nc.dram_tensor("cpack", [128, CPACK_COLS], mm_dtype,
                           kind="ExternalInput").ap()
    outA = nc.dram_tensor("outA", [M2 - 1, npad], F32, kind="ExternalOutput").ap()
    outB = nc.dram_tensor("outB", [M2 - 1, npad], F32, kind="ExternalOutput").ap()

    with tile.TileContext(nc) as tc, ExitStack() as ctx:
        const = ctx.enter_context(tc.tile_pool(name="const", bufs=1))
        hrp = ctx.enter_context(tc.tile_pool(name="hrp", bufs=4))
        h1p = ctx.enter_context(tc.tile_pool(name="h1p", bufs=2))
        epi = ctx.enter_context(tc.tile_pool(name="epi", bufs=3))
        ps1p = ctx.enter_context(tc.tile_pool(name="ps1p", bufs=5, space="PSUM"))
        ps2p = ctx.enter_context(tc.tile_pool(name="ps2p", bufs=2, space="PSUM"))
        bcp = ctx.enter_context(tc.tile_pool(name="bcp", bufs=1, space="PSUM"))

        cph = const.tile([128, HOT_COLS], mm_dtype, tag="cph")
        nc.sync.dma_start(cph[:], cpack[:, 0:HOT_COLS])
        cpc = const.tile([128, COLD_COLS], mm_dtype, tag="cpc")
        # (cold-region DMA is emitted after tile 0's input DMAs below, so the
        # first tile's prefetch isn't queued behind it on the in-order SP)

        def w1a_ap(h, m, slot):
            c = CP_W1A + (h * 2 + m) * 128
            return cph[slot * 64:(slot + 1) * 64, c:c + 128]

        def w1b_ap(h, slot):
            c = CC_W1B + h * 128
            return cpc[slot * 64:(slot + 1) * 64, c:c + 128]

        def w1bf_ap(h, m):
            c = CP_W1BF + (h * 2 + m) * 128
            return cph[:, c:c + 128]

        def w2_ap(kc):
            c = CC_W2 + kc * M2
            return cpc[:, c:c + M2]

        def b1_ap(j):
            return cph[:, CP_B1 + j:CP_B1 + j + 1].bitcast(F32)

        b2_all = cpc[0:M2, CC_B2:CC_B2 + 1].bitcast(F32)
        b2_sig = cpc[0:1, CC_B2:CC_B2 + 1].bitcast(F32)
        ones_ap = cpc[0:1, CC_ONES:CC_ONES + M2]  # mm_dtype, for the bcast matmul

        def epilogue(pend):
            """Masked adds + output DMAs for a finished tile. Emitted one tile
            late so the mask broadcast never waits on the sigmoid, and the
            output DMAs (on gpsimd/SWDGE) never block input prefetch."""
            mask_sb, delt, raw_sb, c0, c1 = pend
            if GP_BCAST:
                bc = epi.tile([M2, tile_n], F32, tag="bc")
                nc.gpsimd.partition_broadcast(bc[:], mask_sb[:], channels=M2)
            else:
                bc = bcp.tile([M2, tile_n], F32, tag="bc")
                nc.tensor.matmul(bc[:], ones_ap, mask_sb[:], start=True, stop=True)
            msk = epi.tile([M2, tile_n], F32, tag="msk")
            nc.vector.tensor_mul(msk[:], delt[:], bc[:])
            outs = epi.tile([M2, tile_n], F32, tag="outs")
            nc.vector.tensor_add(outs[:], msk[:], raw_sb[:])
            nc.gpsimd.dma_start(outA[:, c0:c1], outs[1:M2, :])
            nc.gpsimd.dma_start(outB[:, c0:c1], delt[1:M2, :])

        pend = None
        hr3 = hr.rearrange("p (kc n) -> p kc n", kc=2)
        for t in range(ntiles):
            c0, c1 = t * tile_n, (t + 1) * tile_n

            hrt = hrp.tile([128, 2, tile_n], mm_dtype, tag="hrt")
            nc.sync.dma_start(hrt[:], hr3[:, :, c0:c1])

            raw_sb = epi.tile([M2, tile_n], F32, tag="raw")
            nc.sync.dma_start(raw_sb[:], raw[:, c0:c1])
            if t == 0:
                nc.sync.dma_start(cpc[:], cpack[:, HOT_COLS:CPACK_COLS])

            h1 = h1p.tile([128, K2 * tile_n], mm_dtype, tag="h1")
            ps2 = ps2p.tile([M2, tile_n], F32, tag="ps2")

            def l2_mm(kc):
                nc.tensor.matmul(
                    ps2[:], w2_ap(kc),
                    h1[:, kc * tile_n:(kc + 1) * tile_n],
                    start=(kc == 0), stop=(kc == K2 - 1))

            for h in range(NHEADS):
                ps_m0 = ps1p.tile([128, tile_n], F32, tag="ps1")
                ps_m1 = ps1p.tile([128, tile_n], F32, tag="ps1")
                ps = [ps_m0, ps_m1]
                if row_tiled:
                    # 6 K=64 matmuls per head, strictly alternating T0/T8:
                    # m0-c0@T0, m0-c1@T8, m1-c0@T0, m1-c1@T8, m0-c2@T0, m1-c2@T8
                    for m in range(2):
                        nc.tensor.matmul(ps[m][:], w1a_ap(h, m, 0),
                                         hrt[0:64, 0, :], start=True, stop=False)
                        nc.tensor.matmul(ps[m][:], w1a_ap(h, m, 1),
                                         hrt[64:128, 0, :], start=False, stop=False)
                    for m in range(2):
                        nc.tensor.matmul(ps[m][:], w1b_ap(h, m),
                                         hrt[m * 64:(m + 1) * 64, 1, :],
                                         start=False, stop=True)
                else:
                    # full-array: K=128 chunk + zero-padded K=64 tail chunk
                    for m in range(2):
                        c = CP_W1A + (h * 2 + m) * 128
                        nc.tensor.matmul(ps[m][:], cph[:, c:c + 128],
                                         hrt[:, 0, :], start=True, stop=False)
                        nc.tensor.matmul(ps[m][:], w1bf_ap(h, m), hrt[:, 1, :],
                                         start=False, stop=True)
                for m in range(2):
                    j = 2 * h + m
                    h1s = h1[:, j * tile_n:(j + 1) * tile_n]
                    if m == 0:
                        nc.vector.tensor_scalar(
                            h1s, ps[m][:], b1_ap(j), 0.0, ALU.add, ALU.max)
                    else:
                        nc.scalar.activation(h1s, ps[m][:], AF.Relu,
                                             bias=b1_ap(j))
                # interleave layer-2 matmuls two heads behind their drains so
                # the PE never reaches an h1 block before ACT/DVE finished it
                if h >= 2:
                    l2_mm(2 * (h - 2))
                    l2_mm(2 * (h - 2) + 1)
            for kc in range(2 * (NHEADS - 2), K2):
                l2_mm(kc)

            mask_sb = epi.tile([1, tile_n], F32 if GP_BCAST else mm_dtype,
                               tag="mask")
            nc.scalar.activation(mask_sb[:], ps2[0:1, :], AF.Sigmoid, bias=b2_sig)
            # full-range drain (row 0 = mask logit + bias, unused downstream)
            delt = epi.tile([M2, tile_n], F32, tag="delt")
            nc.scalar.activation(delt[:], ps2[:], AF.Identity, bias=b2_all)

            if pend is not None:
                epilogue(pend)
            pend = (mask_sb, delt, raw_sb, c0, c1)
        epilogue(pend)

    nc.compile()
    return nc


def _time_pos_enc(t):
    i = np.arange(0, 64, 2, dtype=np.float32)
    freq = np.power(np.float32(10000.0), i / np.float32(32.0)).astype(np.float32)
    t = np.float32(t)
    return np.stack([np.sin(t / freq), np.cos(t / freq)], axis=1).reshape(64)


def prep_weights(inputs):
    """Pack weights/biases into the kernel's SBUF-ready layouts.

    The 64 constant tpe rows of the hidden vector are folded into the
    layer-1 bias here (in float64), shrinking the on-chip K to 192."""
    tpe = _time_pos_enc(np.asarray(inputs["time_emb"], np.float32)[0, 0])
    tpe_relu = np.maximum(tpe, 0).astype(np.float64)

    b1k = np.zeros((NHEADS, 2, 128), dtype=np.float32)
    w2stack = np.zeros((K2 * 128, M2), dtype=np.float32)
    b2k = np.zeros((M2, 1), dtype=np.float32)
    cpk = np.zeros((128, CPACK_COLS), dtype=np.float32)
    col = 0
    for hi, hn in enumerate(HEAD_NAMES):
        w1_h = np.asarray(inputs[f"{hn}_w1"], np.float32)
        b1_h = np.asarray(inputs[f"{hn}_b1"], np.float32)
        w2_h = np.asarray(inputs[f"{hn}_w2"], np.float32)
        b2_h = np.asarray(inputs[f"{hn}_b2"], np.float32)
        oh = HEAD_OUT[hi]
        # fold tpe rows (191:255) into the bias, pad K 191->192 and M 255->256
        b1_eff = (b1_h.astype(np.float64)
                  + tpe_relu @ w1_h[KIN:W, :].astype(np.float64))
        w1p = np.zeros((K1P, WP), dtype=np.float32)
        w1p[:KIN, :W] = w1_h[:KIN, :]
        for m in range(2):
            mc = slice(m * 128, (m + 1) * 128)
            cpk[:, CP_W1A + (hi * 2 + m) * 128:
                CP_W1A + (hi * 2 + m + 1) * 128] = w1p[0:128, mc]
            c = HOT_COLS + CC_W1B + hi * 128
            cpk[m * 64:(m + 1) * 64, c:c + 128] = w1p[128:K1P, mc]
            cf = CP_W1BF + (hi * 2 + m) * 128
            cpk[0:64, cf:cf + 128] = w1p[128:K1P, mc]
        b1k[hi] = np.concatenate(
            [b1_eff, [0.0]]).astype(np.float32).reshape(2, 128)
        w2stack[hi * WP:hi * WP + W, col:col + oh] = w2_h
        b2k[col:col + oh, 0] = b2_h
        col += oh
    assert col == M2
    w2_out = np.ascontiguousarray(
        w2stack.reshape(K2, 128, M2).transpose(1, 0, 2).reshape(128, K2 * M2))
    b1_out = np.ascontiguousarray(
        b1k.transpose(2, 0, 1).reshape(128, NHEADS * 2))

    cpk[:, HOT_COLS + CC_W2:HOT_COLS + CC_W2 + K2 * M2] = w2_out
    cpk[:, CP_B1:CP_B1 + 2 * NHEADS] = b1_out
    cpk[0:M2, HOT_COLS + CC_B2] = b2k[:, 0]
    cpk[0, HOT_COLS + CC_ONES + 1:HOT_COLS + CC_ONES + M2] = 1.0
    return cpk


def prep_points(inputs, npad_total):
    """Build relu(hidden) feature-major [256, npad_total] and raw [58, npad_total]."""
    point = np.asarray(inputs["point"], np.float32)
    n = point.shape[0]
    if npad_total is None:
        npad_total = n
    rot = np.asarray(inputs["rotations_input"], np.float32)
    scl = np.asarray(inputs["scales_input"], np.float32)
    opac = np.asarray(inputs["opacity_emb"], np.float32)
    shs = np.asarray(inputs["shs_emb"], np.float32).reshape(n, 48)
    sem = np.asarray(inputs["semantic_feature"], np.float32)
    dx = np.asarray(inputs["dx"], np.float32)
    temb = np.asarray(inputs["time_emb"], np.float32)

    # tpe rows are folded into the layer-1 bias host-side; only the 191
    # per-point features ship to the device (padded to 192)
    hr = np.zeros((K1P, npad_total), dtype=np.float32)
    hr[0:3, :n] = np.maximum(point, 0).T
    hr[3:7, :n] = np.maximum(rot, 0).T
    hr[7:10, :n] = np.maximum(scl, 0).T
    hr[10:11, :n] = np.maximum(opac, 0).T
    hr[11:59, :n] = np.maximum(shs, 0).T
    hr[59:187, :n] = np.maximum(sem, 0).T
    hr[187:190, :n] = np.maximum(dx, 0).T
    hr[190:191, :n] = np.maximum(temb, 0).T

    raw = np.zeros((M2, npad_total), dtype=np.float32)
    raw[1:4, :n] = point.T
    raw[4:7, :n] = scl.T
    raw[7:11, :n] = rot.T
    raw[11:59, :n] = shs.T
    return hr, raw


_CACHED = {}


def _get_nc():
    if "nc" not in _CACHED:
        _CACHED["nc"] = build_bass()
    return _CACHED["nc"]


def make_in_maps(inputs, npad=NPAD, n_cores=N_CORES):
    """Shard + pack full inputs into per-core kernel input dicts."""
    cpk = prep_weights(inputs)
    hr_full, raw_full = prep_points(inputs, None)
    n = hr_full.shape[1]
    n_per_core = n // n_cores
    assert n_per_core * n_cores == n and n_per_core <= npad

    in_maps = []
    for c in range(n_cores):
        s = c * n_per_core
        hrc = np.zeros((K1P, npad), dtype=np.float32)
        hrc[:, :n_per_core] = hr_full[:, s:s + n_per_core]
        rawc = np.zeros((M2, npad), dtype=np.float32)
        rawc[:, :n_per_core] = raw_full[:, s:s + n_per_core]
        # kc=0: hidden rows 0:128; kc=1: rows 128:192 duplicated across both
        # 64-partition halves (feeds the row-tiled K=64 tail matmuls)
        bot = np.concatenate([hrc[128:K1P], hrc[128:K1P]], axis=0)
        hrk = np.ascontiguousarray(
            np.stack([hrc[0:128], bot], axis=1).reshape(128, 2 * npad))
        in_maps.append({"hr": hrk, "raw": rawc, "cpack": cpk})
    return in_maps


def assemble_outputs(inputs, outA, outB, n):
    """outA/outB: [58, n] feature-major device outputs -> reference 7-tuple."""
    pts = np.ascontiguousarray(outA[0:3].T)
    scales = np.ascontiguousarray(outA[3:6].T)
    rotations = np.ascontiguousarray(outA[6:10].T)
    shs_out = np.ascontiguousarray(outA[10:58].T).reshape(n, 16, 3)
    dx_out = np.ascontiguousarray(outB[0:3].T)
    dshs = np.ascontiguousarray(outB[10:58].T).reshape(n, 16, 3)
    opacity = np.asarray(inputs["opacity_emb"], np.float32)[:, :1].copy()
    return (pts, scales, rotations, opacity, shs_out, dx_out, dshs)


def kernel(**inputs):
    import os
    nc = _get_nc()
    in_maps = make_in_maps(inputs)
    trace = bool(int(os.environ.get("KERNEL_TRACE", "0")))

    res = bass_utils.run_bass_kernel_spmd(
        nc, in_maps, core_ids=list(range(N_CORES)), trace=trace)
    _CACHED["last_results"] = res

    outA = np.concatenate(
        [res.results[c]["outA"][:, :N_PER_CORE] for c in range(N_CORES)], axis=1)
    outB = np.concatenate(
        [res.results[c]["outB"][:, :N_PER_CORE] for c in range(N_CORES)], axis=1)
    return assemble_outputs(inputs, outA, outB, N_TOTAL)
